# revision 11
# baseline (speedup 1.0000x reference)
"""GCN graph classifier on 8 Trainium2 NeuronCores (Bass/Tile).

Graphs (and their nodes) are sharded across the 8 cores; each layer's
node table x*dinv@W is replicated via AllGather in bf16.  Message
aggregation = indirect-DMA gathers (degree-class packed) + block-ones
PE matmuls that sum each node's messages on the tensor engine.
GraphNorm stats run as per-graph indirect gathers + ones-vector
matmuls; mean/scale broadcasts come back exactly (f32) through small
indirect gathers.  All index structure is precomputed on the host from
the actual inputs.
"""
import numpy as np
import ml_dtypes

import concourse.bacc as bacc
import concourse.bass as bass
import concourse.mybir as mybir
import concourse.tile as tile
from concourse.bass_utils import run_bass_kernel_spmd

BF = ml_dtypes.bfloat16

N_NODES = 100000
N_GRAPHS = 1024
HID = 32
EPS = 1e-5
NC = 8
P = 128
GPC = N_GRAPHS // NC          # graphs per core
KP_CLASSES = [(12, 10), (16, 8), (21, 6), (25, 5), (32, 4), (64, 2), (128, 1)]
MM_COLS = 16                  # idx-cols (node column-groups) per matmul
GCH = 192                     # idx-cols per gather instruction
LANES = (0, 32, 64)
FG_BANKS = 2                  # psum banks per flush group
LG_MM = FG_BANKS              # matmuls per lane-group
FG_MM = FG_BANKS * 3          # matmuls per flush group (banks x 3 lanes)

f32 = mybir.dt.float32
bf16 = mybir.dt.bfloat16
i32 = mybir.dt.int32


def _preprocess_structure(edge_index, batch):
    ei = np.asarray(edge_index, dtype=np.int64)
    row = np.concatenate([ei[0], np.arange(N_NODES, dtype=np.int64)])
    col = np.concatenate([ei[1], np.arange(N_NODES, dtype=np.int64)])
    batch = np.asarray(batch, dtype=np.int64)

    deg = np.bincount(col, minlength=N_NODES)
    assert deg.max() <= 128, f"max degree {deg.max()} > 128"
    dinv = (1.0 / np.sqrt(np.maximum(deg, 1.0))).astype(np.float32)
    cnt = np.bincount(batch, minlength=N_GRAPHS)
    assert cnt.max() <= 256, f"max graph size {cnt.max()} > 256"
    inv_cnt = (1.0 / np.maximum(cnt, 1.0)).astype(np.float32)

    order = np.argsort(col, kind="stable")
    srcs = row[order]
    indptr = np.zeros(N_NODES + 1, np.int64)
    np.cumsum(np.bincount(col, minlength=N_NODES), out=indptr[1:])

    node_core = (batch // GPC).astype(np.int64)
    core_start = np.searchsorted(batch, np.arange(0, N_GRAPHS + 1, GPC))

    kp_arr = np.array([k for k, _ in KP_CLASSES])
    cls_of = np.searchsorted(kp_arr, deg)

    members = []
    for c in range(NC):
        lo, hi = core_start[c], core_start[c + 1]
        ids = np.arange(lo, hi)
        members.append([ids[cls_of[lo:hi] == k] for k in range(len(KP_CLASSES))])

    cols_k = []
    for k, (kp, npc) in enumerate(KP_CLASSES):
        m = max((len(members[c][k]) + npc - 1) // npc for c in range(NC))
        cols_k.append(-(-m // MM_COLS) * MM_COLS if m else 0)

    mms = []
    for k, (kp, npc) in enumerate(KP_CLASSES):
        for a in range(0, cols_k[k], MM_COLS):
            mms.append((k, a, min(MM_COLS, cols_k[k] - a)))
    n_mm = len(mms)

    # lane-group placement: groups of LG_MM matmuls share agg rows and
    # span FG_BANKS bands; bands advance when rows exhaust.
    descs = []
    R = 0
    B = 0
    for m0 in range(0, n_mm, LG_MM):
        grp = mms[m0:m0 + LG_MM]
        npc_max = max(KP_CLASSES[k][1] for k, _, _ in grp)
        lg = m0 // LG_MM
        lane = LANES[lg % 3]
        fg = lg // 3
        if R + npc_max > P:
            R = 0
            B += FG_BANKS * MM_COLS
        for i, (k, a, ncols) in enumerate(grp):
            descs.append(dict(k=k, a=a, ncols=ncols, npc=KP_CLASSES[k][1],
                              fg=fg, lane=lane, bank=i,
                              aggR=R, band=B + i * MM_COLS))
        R += npc_max
    NX = B + FG_BANKS * MM_COLS
    NX = -(-NX // 4) * 4
    SH = P * NX
    TBL = NC * SH
    CW = sum(cols_k)
    CWP = -(-CW // GCH) * GCH

    agg_p = np.full(N_NODES, -1, np.int32)
    agg_x = np.full(N_NODES, -1, np.int32)
    col_base = np.concatenate([[0], np.cumsum(cols_k)[:-1]]).astype(np.int64)
    for c in range(NC):
        for d in descs:
            k, a, ncols, npc = d["k"], d["a"], d["ncols"], d["npc"]
            mem = members[c][k]
            for j in range(ncols):
                nodes = mem[(a + j) * npc:(a + j + 1) * npc]
                agg_p[nodes] = d["aggR"] + np.arange(len(nodes))
                agg_x[nodes] = d["band"] + j
    r_local = agg_p.astype(np.int64) * NX + agg_x
    r_global = node_core * SH + r_local

    idx_msgs = np.full((NC, P, CWP), TBL, np.int32)
    for c in range(NC):
        for d in descs:
            k, a, ncols, npc = d["k"], d["a"], d["ncols"], d["npc"]
            kp = KP_CLASSES[k][0]
            mem = members[c][k]
            gc0 = col_base[k] + a
            for j in range(ncols):
                nodes = mem[(a + j) * npc:(a + j + 1) * npc]
                for l, v in enumerate(nodes):
                    dv = deg[v]
                    idx_msgs[c, l * kp:l * kp + dv, gc0 + j] = \
                        r_global[srcs[indptr[v]:indptr[v + 1]]]

    # two 128-slot columns per graph (graph sizes can exceed 128)
    idx_stats = np.full((NC, P, 2 * GPC), SH, np.int32)
    for c in range(NC):
        lo, hi = core_start[c], core_start[c + 1]
        b_loc = batch[lo:hi] - c * GPC
        ids = np.arange(lo, hi)
        for g in range(GPC):
            ms = ids[b_loc == g]
            n0 = min(len(ms), P)
            idx_stats[c, :n0, 2 * g] = r_local[ms[:n0]]
            if len(ms) > P:
                idx_stats[c, :len(ms) - P, 2 * g + 1] = r_local[ms[P:]]

    idx_bc = np.full((NC, P, NX), GPC, np.int32)
    dinv_agg = np.zeros((NC, P, NX), np.float32)
    for c in range(NC):
        ids = np.arange(core_start[c], core_start[c + 1])
        idx_bc[c, agg_p[ids], agg_x[ids]] = (batch[ids] - c * GPC).astype(np.int32)
        dinv_agg[c, agg_p[ids], agg_x[ids]] = dinv[ids]

    invcnt_col = inv_cnt.reshape(NC, GPC)

    ones_all = np.zeros((P, sum(n for _, n in KP_CLASSES)), BF)
    ones_off = []
    off = 0
    for kp, npc in KP_CLASSES:
        ones_off.append(off)
        for l in range(npc):
            ones_all[l * kp:(l + 1) * kp, off + l] = 1
        off += npc

    return dict(
        deg=deg, dinv=dinv, inv_cnt=inv_cnt,
        descs=descs, cols_k=cols_k, col_base=col_base, NX=NX, SH=SH,
        TBL=TBL, CW=CW, CWP=CWP,
        r_global=r_global, r_local=r_local, agg_p=agg_p, agg_x=agg_x,
        core_start=core_start,
        idx_msgs=idx_msgs, idx_stats=idx_stats, idx_bc=idx_bc,
        dinv_agg=dinv_agg, invcnt_col=invcnt_col,
        ones_all=ones_all, ones_off=ones_off,
    )


DEBUG = False


def _build(plan):
    NX, SH, TBL, CWP = plan["NX"], plan["SH"], plan["TBL"], plan["CWP"]
    NXF = NX * HID
    NB = NX // 4
    NONES = plan["ones_all"].shape[1]

    nc_ = bacc.Bacc(None, target_bir_lowering=False)

    t1 = nc_.declare_dram_parameter("t1", [TBL + 1, HID], bf16, isOutput=False)
    idxm = nc_.declare_dram_parameter("idxm", [P, CWP], i32, isOutput=False)
    idxs = nc_.declare_dram_parameter("idxs", [P, 2 * GPC], i32, isOutput=False)
    idxb = nc_.declare_dram_parameter("idxb", [P, NX], i32, isOutput=False)
    dinv_in = nc_.declare_dram_parameter("dinv", [P, NX], f32, isOutput=False)
    invc_in = nc_.declare_dram_parameter("invc", [P, 1], f32, isOutput=False)
    ones_in = nc_.declare_dram_parameter("ones", [P, NONES], bf16, isOutput=False)
    prm_in = nc_.declare_dram_parameter("prm", [16, HID], f32, isOutput=False)
    pidx_in = nc_.declare_dram_parameter("pidx", [P, 16], i32, isOutput=False)
    w4_2_in = nc_.declare_dram_parameter("w4_2", [P, P], bf16, isOutput=False)
    w4_3_in = nc_.declare_dram_parameter("w4_3", [P, P], bf16, isOutput=False)
    wl_in = nc_.declare_dram_parameter("wl", [HID, 3], f32, isOutput=False)
    idb_in = nc_.declare_dram_parameter("idb", [P, P], bf16, isOutput=False)
    idf_in = nc_.declare_dram_parameter("idf", [P, P], f32, isOutput=False)
    out_t = nc_.declare_dram_parameter("out", [N_GRAPHS, 3], f32, isOutput=True)
    dbg = {}
    if DEBUG:
        for nm, shp, dt in [("dbg_agg0", [P, NX * HID], f32),
                            ("dbg_conv0", [P, NX * HID], f32),
                            ("dbg_s10", [GPC, HID], f32),
                            ("dbg_cn0", [P, NX * HID], f32),
                            ("dbg_sub0", [P, NX * HID], f32),
                            ("dbg_s20", [GPC, HID], f32),
                            ("dbg_av0", [GPC, HID], f32),
                            ("dbg_x0", [P, NX * HID], f32),
                            ("dbg_gb0", [P, 192 * HID], f32)]:
            dbg[nm] = nc_.declare_dram_parameter(nm, shp, dt, isOutput=True)

    tabA = nc_.dram_tensor("tabA", [TBL + 1, HID], bf16)
    tabB = nc_.dram_tensor("tabB", [TBL + 1, HID], bf16)
    stg = nc_.dram_tensor("stg", [SH, HID], bf16)
    xb1 = nc_.dram_tensor("xb1", [SH + 1, HID], bf16)
    xb2 = nc_.dram_tensor("xb2", [SH + 1, HID], bf16)
    cbd = nc_.dram_tensor("cbd", [GPC + 1, HID], f32)
    abd = nc_.dram_tensor("abd", [GPC + 1, HID], f32)
    srd = nc_.dram_tensor("srd", [2 * GPC, HID], f32)
    lg_in = nc_.dram_tensor("lg_in", [GPC, 3], f32)
    lg_out = nc_.dram_tensor("lg_out", [N_GRAPHS, 3], f32)

    RG = [list(range(NC))]
    AX = mybir.AluOpType
    ACT = mybir.ActivationFunctionType

    with tile.TileContext(nc_) as tc:
        with (
            tc.tile_pool(name="persist", bufs=1) as pp,
            tc.tile_pool(name="work", bufs=1) as wp,
            tc.tile_pool(name="gather", bufs=2) as gp,
            tc.tile_pool(name="stat", bufs=1) as sp,
        ):
            idxm_t = pp.tile([P, CWP], i32)
            nc_.sync.dma_start(out=idxm_t[:], in_=idxm[:, :])
            idxs_t = pp.tile([P, 2 * GPC], i32)
            nc_.sync.dma_start(out=idxs_t[:], in_=idxs[:, :])
            idxb_t = pp.tile([P, NX], i32)
            nc_.sync.dma_start(out=idxb_t[:], in_=idxb[:, :])
            dinv_t = pp.tile([P, NX], f32)
            nc_.sync.dma_start(out=dinv_t[:], in_=dinv_in[:, :])
            invc_t = pp.tile([P, 1], f32)
            nc_.sync.dma_start(out=invc_t[:], in_=invc_in[:, :])
            ones_t = pp.tile([P, NONES], bf16)
            nc_.sync.dma_start(out=ones_t[:], in_=ones_in[:, :])
            w42_t = pp.tile([P, P], bf16)
            nc_.sync.dma_start(out=w42_t[:], in_=w4_2_in[:, :])
            w43_t = pp.tile([P, P], bf16)
            nc_.sync.dma_start(out=w43_t[:], in_=w4_3_in[:, :])
            wl_t = pp.tile([HID, 3], f32)
            nc_.sync.dma_start(out=wl_t[:], in_=wl_in[:, :])
            idb_t = pp.tile([P, P], bf16)
            nc_.sync.dma_start(out=idb_t[:], in_=idb_in[:, :])
            idf_t = pp.tile([P, P], f32)
            nc_.sync.dma_start(out=idf_t[:], in_=idf_in[:, :])
            pidx_t = pp.tile([P, 16], i32)
            nc_.sync.dma_start(out=pidx_t[:], in_=pidx_in[:, :])
            prm_t = pp.tile([P, 16 * HID], f32)
            nc_.gpsimd.indirect_dma_start(
                out=prm_t[:], out_offset=None, in_=prm_in[:],
                in_offset=bass.IndirectOffsetOnAxis(ap=pidx_t[:, :], axis=0))
            ocol_t = pp.tile([P, 1], bf16)
            nc_.vector.memset(ocol_t[:], 1.0)

            zrow = pp.tile([1, HID], f32)
            nc_.vector.memset(zrow[:], 0.0)
            zrow_b = pp.tile([1, HID], bf16)
            nc_.vector.memset(zrow_b[:], 0.0)
            nc_.sync.dma_start(out=tabA[TBL:TBL + 1, :], in_=zrow_b[:])
            nc_.sync.dma_start(out=tabB[TBL:TBL + 1, :], in_=zrow_b[:])
            nc_.sync.dma_start(out=xb1[SH:SH + 1, :], in_=zrow_b[:])
            nc_.sync.dma_start(out=xb2[SH:SH + 1, :], in_=zrow_b[:])
            nc_.sync.dma_start(out=cbd[GPC:GPC + 1, :], in_=zrow[:])
            nc_.sync.dma_start(out=abd[GPC:GPC + 1, :], in_=zrow[:])

            def prm_row(r):
                return prm_t[:, r * HID:(r + 1) * HID]

            def prm_bcast(r):
                return prm_t[:, r * HID:(r + 1) * HID][:, None, :] \
                    .to_broadcast([P, NX, HID])

            def as3(ap):
                return ap.rearrange("p (x f) -> p x f", f=HID)

            def stats_pass(bounce, tag):
                """bounce [SH+1] bf16 -> per-graph sums [GPC, HID] f32."""
                st = sp.tile([P, 2 * GPC * HID], bf16, tag="stbuf")
                for jj in range(2 * GPC):
                    nc_.gpsimd.indirect_dma_start(
                        out=st[:, jj * HID:(jj + 1) * HID],
                        out_offset=None, in_=bounce[:],
                        in_offset=bass.IndirectOffsetOnAxis(
                            ap=idxs_t[:, jj:jj + 1], axis=0))
                with tc.tile_pool(name="psS", bufs=2, space="PSUM") as psS:
                    for m in range(2 * GPC * HID // 512):
                        pss = psS.tile([1, 512], f32, space="PSUM", tag="sps")
                        nc_.tensor.matmul(out=pss[:], lhsT=ocol_t[:],
                                          rhs=st[:, m * 512:(m + 1) * 512],
                                          start=True, stop=True)
                        srow = wp.tile([1, 512], f32, tag="srow")
                        nc_.vector.tensor_copy(out=srow[:], in_=pss[:])
                        nc_.sync.dma_start(
                            out=srd[m * 16:(m + 1) * 16, :], in_=srow[:])
                sw = wp.tile([GPC, 2 * HID], f32, tag=tag + "w")
                nc_.sync.dma_start(
                    out=sw[:],
                    in_=srd[:, :].rearrange("(g two) f -> g (two f)", two=2))
                s = wp.tile([GPC, HID], f32, tag=tag)
                nc_.vector.tensor_tensor(out=s[:], in0=sw[:, 0:HID],
                                         in1=sw[:, HID:2 * HID],
                                         op=mybir.AluOpType.add)
                return s

            def bcast(dram_buf, spread_tile, tag):
                nc_.sync.dma_start(out=dram_buf[0:GPC, :], in_=spread_tile[:])
                outb = wp.tile([P, NXF], f32, tag=tag)
                for jj in range(NX):
                    nc_.gpsimd.indirect_dma_start(
                        out=outb[:, jj * HID:(jj + 1) * HID],
                        out_offset=None, in_=dram_buf[:],
                        in_offset=bass.IndirectOffsetOnAxis(
                            ap=idxb_t[:, jj:jj + 1], axis=0))
                return outb

            descs = plan["descs"]
            col_base = plan["col_base"]
            ones_off = plan["ones_off"]
            tables = [t1, tabA, tabB]
            x_prev = {}

            for L in range(3):
                agg = wp.tile([P, NXF], f32, tag="agg")
                # ---------- aggregation ----------
                gbufs = {}
                with tc.tile_pool(name=f"psA{L}", bufs=2, space="PSUM") as psA:
                    ps = None
                    fg_list = []
                    last_fg = -1
                    for m, d in enumerate(descs):
                        gcol = int(col_base[d["k"]] + d["a"])
                        ch = gcol // GCH
                        if ch not in gbufs:
                            gb = gp.tile([P, GCH * HID], bf16, tag="gbuf")
                            for jj in range(GCH):
                                nc_.gpsimd.indirect_dma_start(
                                    out=gb[:, jj * HID:(jj + 1) * HID],
                                    out_offset=None, in_=tables[L][:],
                                    in_offset=bass.IndirectOffsetOnAxis(
                                        ap=idxm_t[:, ch * GCH + jj:
                                                  ch * GCH + jj + 1],
                                        axis=0))
                            gbufs[ch] = gb
                        if d["fg"] != last_fg:
                            if ps is not None:
                                sc = wp.tile([P, FG_BANKS * 512], f32, tag="scr")
                                nc_.scalar.activation(out=sc[:], in_=ps[:],
                                                      func=ACT.Copy)
                                for dd in fg_list:
                                    nc_.sync.dma_start(
                                        out=agg[dd["aggR"]:dd["aggR"] + dd["npc"],
                                                dd["band"] * HID:
                                                (dd["band"] + dd["ncols"]) * HID],
                                        in_=sc[dd["lane"]:dd["lane"] + dd["npc"],
                                               dd["bank"] * 512:
                                               dd["bank"] * 512 + dd["ncols"] * HID])
                            ps = psA.tile([P, FG_BANKS * 512], f32, space="PSUM",
                                          tag="aggps")
                            fg_list = []
                            last_fg = d["fg"]
                        loc = gcol - ch * GCH
                        off = ones_off[d["k"]]
                        nc_.tensor.matmul(
                            out=ps[d["lane"]:d["lane"] + d["npc"],
                                   d["bank"] * 512:
                                   d["bank"] * 512 + d["ncols"] * HID],
                            lhsT=ones_t[:, off:off + d["npc"]],
                            rhs=gbufs[ch][:, loc * HID:(loc + d["ncols"]) * HID],
                            start=True, stop=True)
                        fg_list.append(d)
                    sc = wp.tile([P, FG_BANKS * 512], f32, tag="scr")
                    nc_.scalar.activation(out=sc[:], in_=ps[:], func=ACT.Copy)
                    for dd in fg_list:
                        nc_.sync.dma_start(
                            out=agg[dd["aggR"]:dd["aggR"] + dd["npc"],
                                    dd["band"] * HID:
                                    (dd["band"] + dd["ncols"]) * HID],
                            in_=sc[dd["lane"]:dd["lane"] + dd["npc"],
                                   dd["bank"] * 512:
                                   dd["bank"] * 512 + dd["ncols"] * HID])

                if DEBUG and L == 0:
                    nc_.sync.dma_start(out=dbg["dbg_agg0"][:, :], in_=agg[:])
                    gbf = wp.tile([P, GCH * HID], f32, tag="gbf")
                    nc_.vector.tensor_copy(out=gbf[:], in_=gbufs[0][:])
                    nc_.sync.dma_start(out=dbg["dbg_gb0"][:, :], in_=gbf[:])
                # ---------- conv = agg * dinv + b ----------
                conv = agg
                nc_.vector.tensor_tensor(
                    out=as3(conv[:]), in0=as3(conv[:]),
                    in1=dinv_t[:][:, :, None].to_broadcast([P, NX, HID]),
                    op=AX.mult)
                nc_.vector.tensor_tensor(
                    out=as3(conv[:]), in0=as3(conv[:]),
                    in1=prm_bcast(0 + L), op=AX.add)

                # ---------- graph norm (two-pass) ----------
                nc_.gpsimd.dma_start(   # cast f32->bf16 in flight
                    out=xb1[0:SH, :].rearrange("(p x) f -> p (x f)", p=P),
                    in_=conv[:])
                s1 = stats_pass(xb1, "s1")
                if DEBUG and L == 0:
                    nc_.sync.dma_start(out=dbg["dbg_conv0"][:, :], in_=conv[:])
                    nc_.sync.dma_start(out=dbg["dbg_s10"][:, :], in_=s1[:])
                cvec = wp.tile([GPC, HID], f32, tag="cvec")
                nc_.vector.tensor_scalar_mul(out=cvec[:], in0=s1[:],
                                             scalar1=invc_t[:, 0:1])
                nc_.vector.tensor_tensor(out=cvec[:], in0=cvec[:],
                                         in1=prm_row(9 + L), op=AX.mult)
                cnode = bcast(cbd, cvec, "bcbuf")
                if DEBUG and L == 0:
                    nc_.sync.dma_start(out=dbg["dbg_cn0"][:, :], in_=cnode[:])
                nc_.vector.tensor_tensor(out=conv[:], in0=conv[:],
                                         in1=cnode[:], op=AX.subtract)
                if DEBUG and L == 0:
                    nc_.sync.dma_start(out=dbg["dbg_sub0"][:, :], in_=conv[:])

                sq = wp.tile([P, NXF], bf16, tag="sq")
                nc_.vector.tensor_tensor(out=sq[:], in0=conv[:], in1=conv[:],
                                         op=AX.mult)
                nc_.sync.dma_start(
                    out=xb2[0:SH, :].rearrange("(p x) f -> p (x f)", p=P),
                    in_=sq[:])
                s2 = stats_pass(xb2, "s2")
                if DEBUG and L == 0:
                    nc_.sync.dma_start(out=dbg["dbg_s20"][:, :], in_=s2[:])
                var = wp.tile([GPC, HID], f32, tag="var")
                nc_.vector.tensor_scalar(out=var[:], in0=s2[:],
                                         scalar1=invc_t[:, 0:1], scalar2=EPS,
                                         op0=AX.mult, op1=AX.add)
                rstd = wp.tile([GPC, HID], f32, tag="rstd")
                nc_.vector.reciprocal(out=rstd[:], in_=var[:])
                nc_.scalar.activation(out=rstd[:], in_=rstd[:], func=ACT.Sqrt)
                avec = wp.tile([GPC, HID], f32, tag="avec")
                nc_.vector.tensor_tensor(out=avec[:], in0=rstd[:],
                                         in1=prm_row(6 + L), op=AX.mult)
                if DEBUG and L == 0:
                    nc_.sync.dma_start(out=dbg["dbg_av0"][:, :], in_=avec[:])
                anode = bcast(abd, avec, "bcbuf")

                nc_.vector.tensor_tensor(out=conv[:], in0=conv[:],
                                         in1=anode[:], op=AX.mult)
                nc_.vector.tensor_tensor(
                    out=as3(conv[:]), in0=as3(conv[:]),
                    in1=prm_bcast(3 + L), op=AX.add)
                if L > 0:
                    nc_.vector.tensor_tensor(out=conv[:], in0=conv[:],
                                             in1=x_prev[L - 1][:], op=AX.add)
                xl = wp.tile([P, NXF], f32, tag=f"x{L % 2}")
                nc_.scalar.activation(out=xl[:], in_=conv[:], func=ACT.Relu)
                if DEBUG and L == 0:
                    nc_.sync.dma_start(out=dbg["dbg_x0"][:, :], in_=xl[:])
                x_prev[L] = xl

                if L < 2:
                    # ---------- next table ----------
                    w4 = w42_t if L == 0 else w43_t
                    tab = tabA if L == 0 else tabB
                    yb = wp.tile([P, NXF], bf16, tag="yb")
                    nc_.vector.tensor_tensor(
                        out=as3(yb[:]), in0=as3(xl[:]),
                        in1=dinv_t[:][:, :, None].to_broadcast([P, NX, HID]),
                        op=AX.mult)
                    xt4 = wp.tile([P, NXF], bf16, tag="xt4")
                    with tc.tile_pool(name=f"psT{L}", bufs=2, space="PSUM") as psT:
                        for b in range(NB):
                            pst = psT.tile([P, P], bf16, space="PSUM", tag="tp")
                            nc_.tensor.transpose(out=pst[:],
                                                 in_=yb[:, b * P:(b + 1) * P],
                                                 identity=idb_t[:])
                            nc_.vector.tensor_copy(
                                out=xt4[:, b * P:(b + 1) * P], in_=pst[:])
                    zt4 = wp.tile([P, NXF], bf16, tag="zt4")
                    with tc.tile_pool(name=f"psW{L}", bufs=2, space="PSUM") as psW:
                        for q in range(NXF // 512):
                            psw = psW.tile([P, 512], f32, space="PSUM", tag="wp")
                            nc_.tensor.matmul(out=psw[:], lhsT=w4[:],
                                              rhs=xt4[:, q * 512:(q + 1) * 512],
                                              start=True, stop=True)
                            nc_.vector.tensor_copy(
                                out=zt4[:, q * 512:(q + 1) * 512], in_=psw[:])
                    zb = wp.tile([P, NXF], bf16, tag="zb")
                    with tc.tile_pool(name=f"psU{L}", bufs=2, space="PSUM") as psU:
                        for b in range(NB):
                            pst = psU.tile([P, P], bf16, space="PSUM", tag="tp2")
                            nc_.tensor.transpose(out=pst[:],
                                                 in_=zt4[:, b * P:(b + 1) * P],
                                                 identity=idb_t[:])
                            nc_.scalar.activation(
                                out=zb[:, b * P:(b + 1) * P], in_=pst[:],
                                func=ACT.Copy)
                    nc_.sync.dma_start(
                        out=stg[:, :].rearrange("(p x) f -> p (x f)", p=P),
                        in_=zb[:])
                    nc_.gpsimd.collective_compute(
                        "AllGather", AX.bypass, replica_groups=RG,
                        ins=[stg[:, :]], outs=[tab[0:TBL, :]])
                else:
                    # ---------- pool + logits ----------
                    x3b = wp.tile([P, NXF], bf16, tag="yb")
                    nc_.vector.tensor_copy(out=x3b[:], in_=xl[:])
                    nc_.sync.dma_start(
                        out=xb1[0:SH, :].rearrange("(p x) f -> p (x f)", p=P),
                        in_=x3b[:])
                    s3 = stats_pass(xb1, "s1")
                    pooled = wp.tile([GPC, HID], f32, tag="pooled")
                    nc_.vector.tensor_scalar_mul(out=pooled[:], in0=s3[:],
                                                 scalar1=invc_t[:, 0:1])
                    with tc.tile_pool(name="psF", bufs=1, space="PSUM") as psF:
                        pstp = psF.tile([HID, P], f32, space="PSUM", tag="pt")
                        nc_.tensor.transpose(out=pstp[:], in_=pooled[:],
                                             identity=idf_t[:])
                        pooledT = wp.tile([HID, P], f32, tag="pooledT")
                        nc_.vector.tensor_copy(out=pooledT[:], in_=pstp[:])
                        psl = psF.tile([P, 3], f32, space="PSUM", tag="lg")
                        nc_.tensor.matmul(out=psl[:], lhsT=pooledT[:],
                                          rhs=wl_t[:], start=True, stop=True)
                        logits = wp.tile([GPC, 3], f32, tag="logits")
                        nc_.vector.tensor_tensor(
                            out=logits[:], in0=psl[:],
                            in1=prm_t[0:GPC, 12 * HID:12 * HID + 3], op=AX.add)
                    nc_.sync.dma_start(out=lg_in[:, :], in_=logits[:])
                    nc_.gpsimd.collective_compute(
                        "AllGather", AX.bypass, replica_groups=RG,
                        ins=[lg_in[:, :]], outs=[lg_out[:, :]])
                    ofin = wp.tile([P, N_GRAPHS * 3 // P], f32, tag="ofin")
                    nc_.sync.dma_start(
                        out=ofin[:],
                        in_=lg_out[:, :].rearrange("(p a) f -> p (a f)", p=P))
                    nc_.sync.dma_start(
                        out=out_t[:, :].rearrange("(p a) f -> p (a f)", p=P),
                        in_=ofin[:])

    nc_.finalize()
    return nc_


_CACHE = {}


def _get_plan_nc(edge_index, batch):
    key = (hash(np.asarray(edge_index)[:, ::997].tobytes()),
           hash(np.asarray(batch)[::97].tobytes()))
    if key not in _CACHE:
        plan = _preprocess_structure(edge_index, batch)
        nc_ = _build(plan)
        _CACHE[key] = (plan, nc_)
    return _CACHE[key]


def kernel(x, edge_index, batch, W1, b1, W2, b2, W3, b3,
           g1, be1, ms1, g2, be2, ms2, g3, be3, ms3, Wl, bl):
    plan, nc_ = _get_plan_nc(edge_index, batch)
    NX, SH, TBL = plan["NX"], plan["SH"], plan["TBL"]
    dinv = plan["dinv"]
    r_global = plan["r_global"]

    x = np.asarray(x, np.float32)
    t1_np = np.zeros((TBL + 1, HID), np.float32)
    t1_np[r_global] = (x @ np.asarray(W1, np.float32)) * dinv[:, None]
    t1_np = t1_np.astype(BF)

    prm_np = np.zeros((16, HID), np.float32)
    for i, v in enumerate([b1, b2, b3, be1, be2, be3, g1, g2, g3,
                           ms1, ms2, ms3]):
        prm_np[i] = np.asarray(v, np.float32)
    prm_np[12, :3] = np.asarray(bl, np.float32)

    def blockdiag(w):
        w4 = np.zeros((P, P), np.float32)
        for s in range(4):
            w4[s * HID:(s + 1) * HID, s * HID:(s + 1) * HID] = \
                np.asarray(w, np.float32)
        return w4.astype(BF)

    in_maps = []
    base = dict(
        t1=t1_np, ones=plan["ones_all"], prm=prm_np,
        pidx=np.tile(np.arange(16, dtype=np.int32), (P, 1)),
        w4_2=blockdiag(W2), w4_3=blockdiag(W3),
        wl=np.asarray(Wl, np.float32),
        idb=np.eye(P, dtype=BF), idf=np.eye(P, dtype=np.float32),
    )
    for c in range(NC):
        in_maps.append(dict(
            base,
            idxm=plan["idx_msgs"][c],
            idxs=plan["idx_stats"][c],
            idxb=plan["idx_bc"][c],
            dinv=plan["dinv_agg"][c],
            invc=plan["invcnt_col"][c][:, None].astype(np.float32),
        ))
    res = run_bass_kernel_spmd(nc_, in_maps, list(range(NC)))
    if DEBUG:
        kernel._last_results = res.results
    return np.asarray(res.results[0]["out"], np.float32)


# revision 13
# speedup vs baseline: 1.0031x; 1.0031x over previous
"""GCN graph classifier on 8 Trainium2 NeuronCores (Bass/Tile).

Graphs (and their nodes) are sharded across the 8 cores; each layer's
node table x*dinv@W is replicated via AllGather in bf16.  Message
aggregation = indirect-DMA gathers (degree-class packed) + block-ones
PE matmuls that sum each node's messages on the tensor engine.
GraphNorm stats run as per-graph indirect gathers + ones-vector
matmuls; mean/scale broadcasts come back exactly (f32) through small
indirect gathers.  All index structure is precomputed on the host from
the actual inputs.
"""
import numpy as np
import ml_dtypes

import concourse.bacc as bacc
import concourse.bass as bass
import concourse.mybir as mybir
import concourse.tile as tile
from concourse.bass_utils import run_bass_kernel_spmd

BF = ml_dtypes.bfloat16

N_NODES = 100000
N_GRAPHS = 1024
HID = 32
EPS = 1e-5
NC = 8
P = 128
GPC = N_GRAPHS // NC          # graphs per core
KP_CLASSES = [(12, 10), (16, 8), (21, 6), (25, 5), (32, 4), (64, 2), (128, 1)]
MM_COLS = 16                  # idx-cols (node column-groups) per matmul
GCH = 192                     # idx-cols per gather instruction
LANES = (0, 32, 64)
FG_BANKS = 2                  # psum banks per flush group
LG_MM = FG_BANKS              # matmuls per lane-group
FG_MM = FG_BANKS * 3          # matmuls per flush group (banks x 3 lanes)

f32 = mybir.dt.float32
bf16 = mybir.dt.bfloat16
i32 = mybir.dt.int32


def _preprocess_structure(edge_index, batch):
    ei = np.asarray(edge_index, dtype=np.int64)
    row = np.concatenate([ei[0], np.arange(N_NODES, dtype=np.int64)])
    col = np.concatenate([ei[1], np.arange(N_NODES, dtype=np.int64)])
    batch = np.asarray(batch, dtype=np.int64)

    deg = np.bincount(col, minlength=N_NODES)
    assert deg.max() <= 128, f"max degree {deg.max()} > 128"
    dinv = (1.0 / np.sqrt(np.maximum(deg, 1.0))).astype(np.float32)
    cnt = np.bincount(batch, minlength=N_GRAPHS)
    assert cnt.max() <= 256, f"max graph size {cnt.max()} > 256"
    inv_cnt = (1.0 / np.maximum(cnt, 1.0)).astype(np.float32)

    order = np.argsort(col, kind="stable")
    srcs = row[order]
    indptr = np.zeros(N_NODES + 1, np.int64)
    np.cumsum(np.bincount(col, minlength=N_NODES), out=indptr[1:])

    node_core = (batch // GPC).astype(np.int64)
    core_start = np.searchsorted(batch, np.arange(0, N_GRAPHS + 1, GPC))

    kp_arr = np.array([k for k, _ in KP_CLASSES])
    cls_of = np.searchsorted(kp_arr, deg)

    members = []
    for c in range(NC):
        lo, hi = core_start[c], core_start[c + 1]
        ids = np.arange(lo, hi)
        members.append([ids[cls_of[lo:hi] == k] for k in range(len(KP_CLASSES))])

    cols_k = []
    for k, (kp, npc) in enumerate(KP_CLASSES):
        m = max((len(members[c][k]) + npc - 1) // npc for c in range(NC))
        cols_k.append(-(-m // MM_COLS) * MM_COLS if m else 0)

    mms = []
    for k, (kp, npc) in enumerate(KP_CLASSES):
        for a in range(0, cols_k[k], MM_COLS):
            mms.append((k, a, min(MM_COLS, cols_k[k] - a)))
    n_mm = len(mms)

    # lane-group placement: groups of LG_MM matmuls share agg rows and
    # span FG_BANKS bands; bands advance when rows exhaust.
    descs = []
    R = 0
    B = 0
    for m0 in range(0, n_mm, LG_MM):
        grp = mms[m0:m0 + LG_MM]
        npc_max = max(KP_CLASSES[k][1] for k, _, _ in grp)
        lg = m0 // LG_MM
        lane = LANES[lg % 3]
        fg = lg // 3
        if R + npc_max > P:
            R = 0
            B += FG_BANKS * MM_COLS
        for i, (k, a, ncols) in enumerate(grp):
            descs.append(dict(k=k, a=a, ncols=ncols, npc=KP_CLASSES[k][1],
                              fg=fg, lane=lane, bank=i,
                              aggR=R, band=B + i * MM_COLS))
        R += npc_max
    NX = B + FG_BANKS * MM_COLS
    NX = -(-NX // 4) * 4
    SH = P * NX
    TBL = NC * SH
    CW = sum(cols_k)
    CWP = -(-CW // GCH) * GCH

    agg_p = np.full(N_NODES, -1, np.int32)
    agg_x = np.full(N_NODES, -1, np.int32)
    col_base = np.concatenate([[0], np.cumsum(cols_k)[:-1]]).astype(np.int64)
    for c in range(NC):
        for d in descs:
            k, a, ncols, npc = d["k"], d["a"], d["ncols"], d["npc"]
            mem = members[c][k]
            for j in range(ncols):
                nodes = mem[(a + j) * npc:(a + j + 1) * npc]
                agg_p[nodes] = d["aggR"] + np.arange(len(nodes))
                agg_x[nodes] = d["band"] + j
    r_local = agg_p.astype(np.int64) * NX + agg_x
    r_global = node_core * SH + r_local

    idx_msgs = np.full((NC, P, CWP), TBL, np.int32)
    for c in range(NC):
        for d in descs:
            k, a, ncols, npc = d["k"], d["a"], d["ncols"], d["npc"]
            kp = KP_CLASSES[k][0]
            mem = members[c][k]
            gc0 = col_base[k] + a
            for j in range(ncols):
                nodes = mem[(a + j) * npc:(a + j + 1) * npc]
                for l, v in enumerate(nodes):
                    dv = deg[v]
                    idx_msgs[c, l * kp:l * kp + dv, gc0 + j] = \
                        r_global[srcs[indptr[v]:indptr[v + 1]]]

    # two 128-slot columns per graph (graph sizes can exceed 128)
    idx_stats = np.full((NC, P, 2 * GPC), SH, np.int32)
    for c in range(NC):
        lo, hi = core_start[c], core_start[c + 1]
        b_loc = batch[lo:hi] - c * GPC
        ids = np.arange(lo, hi)
        for g in range(GPC):
            ms = ids[b_loc == g]
            n0 = min(len(ms), P)
            idx_stats[c, :n0, 2 * g] = r_local[ms[:n0]]
            if len(ms) > P:
                idx_stats[c, :len(ms) - P, 2 * g + 1] = r_local[ms[P:]]

    idx_bc = np.full((NC, P, NX), GPC, np.int32)
    dinv_agg = np.zeros((NC, P, NX), np.float32)
    for c in range(NC):
        ids = np.arange(core_start[c], core_start[c + 1])
        idx_bc[c, agg_p[ids], agg_x[ids]] = (batch[ids] - c * GPC).astype(np.int32)
        dinv_agg[c, agg_p[ids], agg_x[ids]] = dinv[ids]

    invcnt_col = inv_cnt.reshape(NC, GPC)

    ones_all = np.zeros((P, sum(n for _, n in KP_CLASSES)), BF)
    ones_off = []
    off = 0
    for kp, npc in KP_CLASSES:
        ones_off.append(off)
        for l in range(npc):
            ones_all[l * kp:(l + 1) * kp, off + l] = 1
        off += npc

    msg_col_real = (idx_msgs != TBL).any(axis=(0, 1))        # [CWP]
    stat_col_real = (idx_stats != SH).any(axis=(0, 1))       # [2*GPC]

    return dict(
        msg_col_real=msg_col_real, stat_col_real=stat_col_real,
        deg=deg, dinv=dinv, inv_cnt=inv_cnt,
        descs=descs, cols_k=cols_k, col_base=col_base, NX=NX, SH=SH,
        TBL=TBL, CW=CW, CWP=CWP,
        r_global=r_global, r_local=r_local, agg_p=agg_p, agg_x=agg_x,
        core_start=core_start,
        idx_msgs=idx_msgs, idx_stats=idx_stats, idx_bc=idx_bc,
        dinv_agg=dinv_agg, invcnt_col=invcnt_col,
        ones_all=ones_all, ones_off=ones_off,
    )


DEBUG = False


def _build(plan):
    NX, SH, TBL, CWP = plan["NX"], plan["SH"], plan["TBL"], plan["CWP"]
    NXF = NX * HID
    NB = NX // 4
    NONES = plan["ones_all"].shape[1]

    nc_ = bacc.Bacc(None, target_bir_lowering=False)

    t1 = nc_.declare_dram_parameter("t1", [TBL + 1, HID], bf16, isOutput=False)
    idxm = nc_.declare_dram_parameter("idxm", [P, CWP], i32, isOutput=False)
    idxs = nc_.declare_dram_parameter("idxs", [P, 2 * GPC], i32, isOutput=False)
    idxb = nc_.declare_dram_parameter("idxb", [P, NX], i32, isOutput=False)
    dinv_in = nc_.declare_dram_parameter("dinv", [P, NX], f32, isOutput=False)
    invc_in = nc_.declare_dram_parameter("invc", [P, 1], f32, isOutput=False)
    ones_in = nc_.declare_dram_parameter("ones", [P, NONES], bf16, isOutput=False)
    prm_in = nc_.declare_dram_parameter("prm", [16, HID], f32, isOutput=False)
    pidx_in = nc_.declare_dram_parameter("pidx", [P, 16], i32, isOutput=False)
    w4_2_in = nc_.declare_dram_parameter("w4_2", [P, P], bf16, isOutput=False)
    w4_3_in = nc_.declare_dram_parameter("w4_3", [P, P], bf16, isOutput=False)
    wl_in = nc_.declare_dram_parameter("wl", [HID, 3], f32, isOutput=False)
    idb_in = nc_.declare_dram_parameter("idb", [P, P], bf16, isOutput=False)
    idf_in = nc_.declare_dram_parameter("idf", [P, P], f32, isOutput=False)
    out_t = nc_.declare_dram_parameter("out", [N_GRAPHS, 3], f32, isOutput=True)
    dbg = {}
    if DEBUG:
        for nm, shp, dt in [("dbg_agg0", [P, NX * HID], f32),
                            ("dbg_conv0", [P, NX * HID], f32),
                            ("dbg_s10", [GPC, HID], f32),
                            ("dbg_cn0", [P, NX * HID], f32),
                            ("dbg_sub0", [P, NX * HID], f32),
                            ("dbg_s20", [GPC, HID], f32),
                            ("dbg_av0", [GPC, HID], f32),
                            ("dbg_x0", [P, NX * HID], f32),
                            ("dbg_gb0", [P, 192 * HID], f32)]:
            dbg[nm] = nc_.declare_dram_parameter(nm, shp, dt, isOutput=True)

    tabA = nc_.dram_tensor("tabA", [TBL + 1, HID], bf16)
    tabB = nc_.dram_tensor("tabB", [TBL + 1, HID], bf16)
    stg = nc_.dram_tensor("stg", [SH, HID], bf16)
    xb1 = nc_.dram_tensor("xb1", [SH + 1, HID], bf16)
    xb2 = nc_.dram_tensor("xb2", [SH + 1, HID], bf16)
    cbd = nc_.dram_tensor("cbd", [GPC + 1, HID], f32)
    abd = nc_.dram_tensor("abd", [GPC + 1, HID], f32)
    srd = nc_.dram_tensor("srd", [2 * GPC, HID], f32)
    lg_in = nc_.dram_tensor("lg_in", [GPC, 3], f32)
    lg_out = nc_.dram_tensor("lg_out", [N_GRAPHS, 3], f32)

    RG = [list(range(NC))]
    AX = mybir.AluOpType
    ACT = mybir.ActivationFunctionType

    with tile.TileContext(nc_) as tc:
        with (
            tc.tile_pool(name="persist", bufs=1) as pp,
            tc.tile_pool(name="work", bufs=1) as wp,
            tc.tile_pool(name="gather", bufs=2) as gp,
            tc.tile_pool(name="stat", bufs=1) as sp,
        ):
            idxm_t = pp.tile([P, CWP], i32)
            nc_.sync.dma_start(out=idxm_t[:], in_=idxm[:, :])
            idxs_t = pp.tile([P, 2 * GPC], i32)
            nc_.sync.dma_start(out=idxs_t[:], in_=idxs[:, :])
            idxb_t = pp.tile([P, NX], i32)
            nc_.sync.dma_start(out=idxb_t[:], in_=idxb[:, :])
            dinv_t = pp.tile([P, NX], f32)
            nc_.sync.dma_start(out=dinv_t[:], in_=dinv_in[:, :])
            invc_t = pp.tile([P, 1], f32)
            nc_.sync.dma_start(out=invc_t[:], in_=invc_in[:, :])
            ones_t = pp.tile([P, NONES], bf16)
            nc_.sync.dma_start(out=ones_t[:], in_=ones_in[:, :])
            w42_t = pp.tile([P, P], bf16)
            nc_.sync.dma_start(out=w42_t[:], in_=w4_2_in[:, :])
            w43_t = pp.tile([P, P], bf16)
            nc_.sync.dma_start(out=w43_t[:], in_=w4_3_in[:, :])
            wl_t = pp.tile([HID, 3], f32)
            nc_.sync.dma_start(out=wl_t[:], in_=wl_in[:, :])
            idb_t = pp.tile([P, P], bf16)
            nc_.sync.dma_start(out=idb_t[:], in_=idb_in[:, :])
            idf_t = pp.tile([P, P], f32)
            nc_.sync.dma_start(out=idf_t[:], in_=idf_in[:, :])
            pidx_t = pp.tile([P, 16], i32)
            nc_.sync.dma_start(out=pidx_t[:], in_=pidx_in[:, :])
            prm_t = pp.tile([P, 16 * HID], f32)
            nc_.gpsimd.indirect_dma_start(
                out=prm_t[:], out_offset=None, in_=prm_in[:],
                in_offset=bass.IndirectOffsetOnAxis(ap=pidx_t[:, :], axis=0))
            ocol_t = pp.tile([P, 1], bf16)
            nc_.vector.memset(ocol_t[:], 1.0)

            zrow = pp.tile([1, HID], f32)
            nc_.vector.memset(zrow[:], 0.0)
            zrow_b = pp.tile([1, HID], bf16)
            nc_.vector.memset(zrow_b[:], 0.0)
            nc_.sync.dma_start(out=tabA[TBL:TBL + 1, :], in_=zrow_b[:])
            nc_.sync.dma_start(out=tabB[TBL:TBL + 1, :], in_=zrow_b[:])
            nc_.sync.dma_start(out=xb1[SH:SH + 1, :], in_=zrow_b[:])
            nc_.sync.dma_start(out=xb2[SH:SH + 1, :], in_=zrow_b[:])
            nc_.sync.dma_start(out=cbd[GPC:GPC + 1, :], in_=zrow[:])
            nc_.sync.dma_start(out=abd[GPC:GPC + 1, :], in_=zrow[:])

            def prm_row(r):
                return prm_t[:, r * HID:(r + 1) * HID]

            def prm_bcast(r):
                return prm_t[:, r * HID:(r + 1) * HID][:, None, :] \
                    .to_broadcast([P, NX, HID])

            def as3(ap):
                return ap.rearrange("p (x f) -> p x f", f=HID)

            def stats_pass(bounce, tag):
                """bounce [SH+1] bf16 -> per-graph sums [GPC, HID] f32."""
                st = sp.tile([P, 2 * GPC * HID], bf16, tag="stbuf")
                for jj in range(2 * GPC):
                    nc_.gpsimd.indirect_dma_start(
                        out=st[:, jj * HID:(jj + 1) * HID],
                        out_offset=None, in_=bounce[:],
                        in_offset=bass.IndirectOffsetOnAxis(
                            ap=idxs_t[:, jj:jj + 1], axis=0))
                with tc.tile_pool(name="psS", bufs=2, space="PSUM") as psS:
                    for m in range(2 * GPC * HID // 512):
                        pss = psS.tile([1, 512], f32, space="PSUM", tag="sps")
                        nc_.tensor.matmul(out=pss[:], lhsT=ocol_t[:],
                                          rhs=st[:, m * 512:(m + 1) * 512],
                                          start=True, stop=True)
                        srow = wp.tile([1, 512], f32, tag="srow")
                        nc_.vector.tensor_copy(out=srow[:], in_=pss[:])
                        nc_.sync.dma_start(
                            out=srd[m * 16:(m + 1) * 16, :], in_=srow[:])
                sw = wp.tile([GPC, 2 * HID], f32, tag=tag + "w")
                nc_.sync.dma_start(
                    out=sw[:],
                    in_=srd[:, :].rearrange("(g two) f -> g (two f)", two=2))
                s = wp.tile([GPC, HID], f32, tag=tag)
                nc_.vector.tensor_tensor(out=s[:], in0=sw[:, 0:HID],
                                         in1=sw[:, HID:2 * HID],
                                         op=mybir.AluOpType.add)
                return s

            def bcast(dram_buf, spread_tile, tag):
                nc_.sync.dma_start(out=dram_buf[0:GPC, :], in_=spread_tile[:])
                outb = wp.tile([P, NXF], f32, tag=tag)
                for jj in range(NX):
                    nc_.gpsimd.indirect_dma_start(
                        out=outb[:, jj * HID:(jj + 1) * HID],
                        out_offset=None, in_=dram_buf[:],
                        in_offset=bass.IndirectOffsetOnAxis(
                            ap=idxb_t[:, jj:jj + 1], axis=0))
                return outb

            descs = plan["descs"]
            col_base = plan["col_base"]
            ones_off = plan["ones_off"]
            tables = [t1, tabA, tabB]
            x_prev = {}

            for L in range(3):
                agg = wp.tile([P, NXF], f32, tag="agg")
                # ---------- aggregation ----------
                gbufs = {}
                with tc.tile_pool(name=f"psA{L}", bufs=2, space="PSUM") as psA:
                    ps = None
                    fg_list = []
                    last_fg = -1
                    for m, d in enumerate(descs):
                        gcol = int(col_base[d["k"]] + d["a"])
                        ch = gcol // GCH
                        if ch not in gbufs:
                            gb = gp.tile([P, GCH * HID], bf16, tag="gbuf")
                            for jj in range(GCH):
                                nc_.gpsimd.indirect_dma_start(
                                    out=gb[:, jj * HID:(jj + 1) * HID],
                                    out_offset=None, in_=tables[L][:],
                                    in_offset=bass.IndirectOffsetOnAxis(
                                        ap=idxm_t[:, ch * GCH + jj:
                                                  ch * GCH + jj + 1],
                                        axis=0))
                            gbufs[ch] = gb
                        if d["fg"] != last_fg:
                            if ps is not None:
                                sc = wp.tile([P, FG_BANKS * 512], f32, tag="scr")
                                nc_.scalar.activation(out=sc[:], in_=ps[:],
                                                      func=ACT.Copy)
                                for dd in fg_list:
                                    nc_.sync.dma_start(
                                        out=agg[dd["aggR"]:dd["aggR"] + dd["npc"],
                                                dd["band"] * HID:
                                                (dd["band"] + dd["ncols"]) * HID],
                                        in_=sc[dd["lane"]:dd["lane"] + dd["npc"],
                                               dd["bank"] * 512:
                                               dd["bank"] * 512 + dd["ncols"] * HID])
                            ps = psA.tile([P, FG_BANKS * 512], f32, space="PSUM",
                                          tag="aggps")
                            fg_list = []
                            last_fg = d["fg"]
                        loc = gcol - ch * GCH
                        off = ones_off[d["k"]]
                        nc_.tensor.matmul(
                            out=ps[d["lane"]:d["lane"] + d["npc"],
                                   d["bank"] * 512:
                                   d["bank"] * 512 + d["ncols"] * HID],
                            lhsT=ones_t[:, off:off + d["npc"]],
                            rhs=gbufs[ch][:, loc * HID:(loc + d["ncols"]) * HID],
                            start=True, stop=True)
                        fg_list.append(d)
                    sc = wp.tile([P, FG_BANKS * 512], f32, tag="scr")
                    nc_.scalar.activation(out=sc[:], in_=ps[:], func=ACT.Copy)
                    for dd in fg_list:
                        nc_.sync.dma_start(
                            out=agg[dd["aggR"]:dd["aggR"] + dd["npc"],
                                    dd["band"] * HID:
                                    (dd["band"] + dd["ncols"]) * HID],
                            in_=sc[dd["lane"]:dd["lane"] + dd["npc"],
                                   dd["bank"] * 512:
                                   dd["bank"] * 512 + dd["ncols"] * HID])

                if DEBUG and L == 0:
                    nc_.sync.dma_start(out=dbg["dbg_agg0"][:, :], in_=agg[:])
                    gbf = wp.tile([P, GCH * HID], f32, tag="gbf")
                    nc_.vector.tensor_copy(out=gbf[:], in_=gbufs[0][:])
                    nc_.sync.dma_start(out=dbg["dbg_gb0"][:, :], in_=gbf[:])
                # ---------- conv = agg * dinv + b ----------
                conv = agg
                nc_.vector.tensor_tensor(
                    out=as3(conv[:]), in0=as3(conv[:]),
                    in1=dinv_t[:][:, :, None].to_broadcast([P, NX, HID]),
                    op=AX.mult)
                nc_.vector.tensor_tensor(
                    out=as3(conv[:]), in0=as3(conv[:]),
                    in1=prm_bcast(0 + L), op=AX.add)

                # ---------- graph norm (two-pass) ----------
                nc_.gpsimd.dma_start(   # cast f32->bf16 in flight
                    out=xb1[0:SH, :].rearrange("(p x) f -> p (x f)", p=P),
                    in_=conv[:])
                s1 = stats_pass(xb1, "s1")
                if DEBUG and L == 0:
                    nc_.sync.dma_start(out=dbg["dbg_conv0"][:, :], in_=conv[:])
                    nc_.sync.dma_start(out=dbg["dbg_s10"][:, :], in_=s1[:])
                cvec = wp.tile([GPC, HID], f32, tag="cvec")
                nc_.vector.tensor_scalar_mul(out=cvec[:], in0=s1[:],
                                             scalar1=invc_t[:, 0:1])
                nc_.vector.tensor_tensor(out=cvec[:], in0=cvec[:],
                                         in1=prm_row(9 + L), op=AX.mult)
                cnode = bcast(cbd, cvec, "bcbuf")
                if DEBUG and L == 0:
                    nc_.sync.dma_start(out=dbg["dbg_cn0"][:, :], in_=cnode[:])
                nc_.vector.tensor_tensor(out=conv[:], in0=conv[:],
                                         in1=cnode[:], op=AX.subtract)
                if DEBUG and L == 0:
                    nc_.sync.dma_start(out=dbg["dbg_sub0"][:, :], in_=conv[:])

                sq = wp.tile([P, NXF], bf16, tag="sq")
                nc_.vector.tensor_tensor(out=sq[:], in0=conv[:], in1=conv[:],
                                         op=AX.mult)
                nc_.sync.dma_start(
                    out=xb2[0:SH, :].rearrange("(p x) f -> p (x f)", p=P),
                    in_=sq[:])
                s2 = stats_pass(xb2, "s2")
                if DEBUG and L == 0:
                    nc_.sync.dma_start(out=dbg["dbg_s20"][:, :], in_=s2[:])
                var = wp.tile([GPC, HID], f32, tag="var")
                nc_.vector.tensor_scalar(out=var[:], in0=s2[:],
                                         scalar1=invc_t[:, 0:1], scalar2=EPS,
                                         op0=AX.mult, op1=AX.add)
                rstd = wp.tile([GPC, HID], f32, tag="rstd")
                nc_.vector.reciprocal(out=rstd[:], in_=var[:])
                nc_.scalar.activation(out=rstd[:], in_=rstd[:], func=ACT.Sqrt)
                avec = wp.tile([GPC, HID], f32, tag="avec")
                nc_.vector.tensor_tensor(out=avec[:], in0=rstd[:],
                                         in1=prm_row(6 + L), op=AX.mult)
                if DEBUG and L == 0:
                    nc_.sync.dma_start(out=dbg["dbg_av0"][:, :], in_=avec[:])
                anode = bcast(abd, avec, "bcbuf")

                nc_.vector.tensor_tensor(out=conv[:], in0=conv[:],
                                         in1=anode[:], op=AX.mult)
                nc_.vector.tensor_tensor(
                    out=as3(conv[:]), in0=as3(conv[:]),
                    in1=prm_bcast(3 + L), op=AX.add)
                if L > 0:
                    nc_.vector.tensor_tensor(out=conv[:], in0=conv[:],
                                             in1=x_prev[L - 1][:], op=AX.add)
                xl = wp.tile([P, NXF], f32, tag=f"x{L % 2}")
                nc_.scalar.activation(out=xl[:], in_=conv[:], func=ACT.Relu)
                if DEBUG and L == 0:
                    nc_.sync.dma_start(out=dbg["dbg_x0"][:, :], in_=xl[:])
                x_prev[L] = xl

                if L < 2:
                    # ---------- next table ----------
                    w4 = w42_t if L == 0 else w43_t
                    tab = tabA if L == 0 else tabB
                    yb = wp.tile([P, NXF], bf16, tag="yb")
                    nc_.vector.tensor_tensor(
                        out=as3(yb[:]), in0=as3(xl[:]),
                        in1=dinv_t[:][:, :, None].to_broadcast([P, NX, HID]),
                        op=AX.mult)
                    xt4 = wp.tile([P, NXF], bf16, tag="xt4")
                    with tc.tile_pool(name=f"psT{L}", bufs=2, space="PSUM") as psT:
                        for b in range(NB):
                            pst = psT.tile([P, P], bf16, space="PSUM", tag="tp")
                            nc_.tensor.transpose(out=pst[:],
                                                 in_=yb[:, b * P:(b + 1) * P],
                                                 identity=idb_t[:])
                            nc_.vector.tensor_copy(
                                out=xt4[:, b * P:(b + 1) * P], in_=pst[:])
                    zt4 = wp.tile([P, NXF], bf16, tag="zt4")
                    with tc.tile_pool(name=f"psW{L}", bufs=2, space="PSUM") as psW:
                        for q in range(NXF // 512):
                            psw = psW.tile([P, 512], f32, space="PSUM", tag="wp")
                            nc_.tensor.matmul(out=psw[:], lhsT=w4[:],
                                              rhs=xt4[:, q * 512:(q + 1) * 512],
                                              start=True, stop=True)
                            nc_.vector.tensor_copy(
                                out=zt4[:, q * 512:(q + 1) * 512], in_=psw[:])
                    zb = wp.tile([P, NXF], bf16, tag="zb")
                    with tc.tile_pool(name=f"psU{L}", bufs=2, space="PSUM") as psU:
                        for b in range(NB):
                            pst = psU.tile([P, P], bf16, space="PSUM", tag="tp2")
                            nc_.tensor.transpose(out=pst[:],
                                                 in_=zt4[:, b * P:(b + 1) * P],
                                                 identity=idb_t[:])
                            nc_.scalar.activation(
                                out=zb[:, b * P:(b + 1) * P], in_=pst[:],
                                func=ACT.Copy)
                    nc_.sync.dma_start(
                        out=stg[:, :].rearrange("(p x) f -> p (x f)", p=P),
                        in_=zb[:])
                    nc_.gpsimd.collective_compute(
                        "AllGather", AX.bypass, replica_groups=RG,
                        ins=[stg[:, :]], outs=[tab[0:TBL, :]])
                else:
                    # ---------- pool + logits ----------
                    x3b = wp.tile([P, NXF], bf16, tag="yb")
                    nc_.vector.tensor_copy(out=x3b[:], in_=xl[:])
                    nc_.sync.dma_start(
                        out=xb1[0:SH, :].rearrange("(p x) f -> p (x f)", p=P),
                        in_=x3b[:])
                    s3 = stats_pass(xb1, "s1")
                    pooled = wp.tile([GPC, HID], f32, tag="pooled")
                    nc_.vector.tensor_scalar_mul(out=pooled[:], in0=s3[:],
                                                 scalar1=invc_t[:, 0:1])
                    with tc.tile_pool(name="psF", bufs=1, space="PSUM") as psF:
                        pstp = psF.tile([HID, P], f32, space="PSUM", tag="pt")
                        nc_.tensor.transpose(out=pstp[:], in_=pooled[:],
                                             identity=idf_t[:])
                        pooledT = wp.tile([HID, P], f32, tag="pooledT")
                        nc_.vector.tensor_copy(out=pooledT[:], in_=pstp[:])
                        psl = psF.tile([P, 3], f32, space="PSUM", tag="lg")
                        nc_.tensor.matmul(out=psl[:], lhsT=pooledT[:],
                                          rhs=wl_t[:], start=True, stop=True)
                        logits = wp.tile([GPC, 3], f32, tag="logits")
                        nc_.vector.tensor_tensor(
                            out=logits[:], in0=psl[:],
                            in1=prm_t[0:GPC, 12 * HID:12 * HID + 3], op=AX.add)
                    nc_.sync.dma_start(out=lg_in[:, :], in_=logits[:])
                    nc_.gpsimd.collective_compute(
                        "AllGather", AX.bypass, replica_groups=RG,
                        ins=[lg_in[:, :]], outs=[lg_out[:, :]])
                    ofin = wp.tile([P, N_GRAPHS * 3 // P], f32, tag="ofin")
                    nc_.sync.dma_start(
                        out=ofin[:],
                        in_=lg_out[:, :].rearrange("(p a) f -> p (a f)", p=P))
                    nc_.sync.dma_start(
                        out=out_t[:, :].rearrange("(p a) f -> p (a f)", p=P),
                        in_=ofin[:])

    nc_.finalize()
    return nc_


_CACHE = {}


def _get_plan_nc(edge_index, batch):
    key = (hash(np.asarray(edge_index)[:, ::997].tobytes()),
           hash(np.asarray(batch)[::97].tobytes()))
    if key not in _CACHE:
        plan = _preprocess_structure(edge_index, batch)
        nc_ = _build(plan)
        _CACHE[key] = (plan, nc_)
    return _CACHE[key]


def kernel(x, edge_index, batch, W1, b1, W2, b2, W3, b3,
           g1, be1, ms1, g2, be2, ms2, g3, be3, ms3, Wl, bl):
    plan, nc_ = _get_plan_nc(edge_index, batch)
    NX, SH, TBL = plan["NX"], plan["SH"], plan["TBL"]
    dinv = plan["dinv"]
    r_global = plan["r_global"]

    x = np.asarray(x, np.float32)
    t1_np = np.zeros((TBL + 1, HID), np.float32)
    t1_np[r_global] = (x @ np.asarray(W1, np.float32)) * dinv[:, None]
    t1_np = t1_np.astype(BF)

    prm_np = np.zeros((16, HID), np.float32)
    for i, v in enumerate([b1, b2, b3, be1, be2, be3, g1, g2, g3,
                           ms1, ms2, ms3]):
        prm_np[i] = np.asarray(v, np.float32)
    prm_np[12, :3] = np.asarray(bl, np.float32)

    def blockdiag(w):
        w4 = np.zeros((P, P), np.float32)
        for s in range(4):
            w4[s * HID:(s + 1) * HID, s * HID:(s + 1) * HID] = \
                np.asarray(w, np.float32)
        return w4.astype(BF)

    in_maps = []
    base = dict(
        t1=t1_np, ones=plan["ones_all"], prm=prm_np,
        pidx=np.tile(np.arange(16, dtype=np.int32), (P, 1)),
        w4_2=blockdiag(W2), w4_3=blockdiag(W3),
        wl=np.asarray(Wl, np.float32),
        idb=np.eye(P, dtype=BF), idf=np.eye(P, dtype=np.float32),
    )
    for c in range(NC):
        in_maps.append(dict(
            base,
            idxm=plan["idx_msgs"][c],
            idxs=plan["idx_stats"][c],
            idxb=plan["idx_bc"][c],
            dinv=plan["dinv_agg"][c],
            invc=plan["invcnt_col"][c][:, None].astype(np.float32),
        ))
    res = run_bass_kernel_spmd(nc_, in_maps, list(range(NC)))
    if DEBUG:
        kernel._last_results = res.results
    return np.asarray(res.results[0]["out"], np.float32)
